# revision 5
# baseline (speedup 1.0000x reference)
"""Trainium2 Bass kernel for ExpertsChooseExpand MoE routing.

Problem (per batch b):
    y[e,c,:] = W_e @ x[b,e,c,:] + bias          # per-expert GEMM
    out[b,t,:] = sum_{(e,c): idx[b,e,c]==t} gate[b,e,c] * y[e,c,:]

Strategy: data-parallel over batch B=8 -> one batch per NeuronCore, no
collectives. Per core:
  P1: gate-scale x rows (DVE), per-expert GEMM (PE), add gate-scaled bias
      on PSUM eviction (DVE), then an indirect DMA row-scatter writes
      each contribution row to its slot in a bin-padded token-sorted
      buffer (host argsorts the indices; each 128-token bin owns a fixed
      256-row slot, so the kernel structure is data-independent).
  P2: the padded sorted rows are re-read as 128-row K-tiles (2 per
      bin); a 0/1 "is_equal" onehot built on the DVE (sorted tokens vs
      an iota ramp) feeds 2 accumulating matmuls per bin that
      segment-sum the bin into PSUM; finished bins stream out.
Pad slots rely on the PJRT zero-donated output buffer (outputs start
zeroed), and their token entries are -1 so the onehot coefficient is 0.
All arithmetic is f32; result matches the reference to fp rounding.
"""
import numpy as np

B, E, C, I, O, T = 8, 8, 1024, 128, 256, 8192
K = E * C          # contribution rows per batch
NT = K // 128      # 64 k-tiles
NBIN = T // 128    # 64 token bins
KPAD = 256         # padded rows per bin
NPT = NBIN * KPAD // 128   # padded k-tiles (128)
N_CORES = 8

LAST_EXEC_NS = None
LAST_RESULT = None

# ----------------------------------------------------------------------
# Environment patches (this container's walrus accepts at most 1 sem wait
# per instruction; TileContext's tail drain and scheduler can attach
# more). Applied once at import.
# ----------------------------------------------------------------------
_PATCHED = False


def _install_patches():
    global _PATCHED
    if _PATCHED:
        return
    import bass_rust
    import concourse.tile as tile
    from concourse.vector_clock import ScopedClock

    _OP_FOR_MODE = {
        "sem-ge-imm": "sem-ge",
        "sem-eq-imm": "sem-eq",
        "sem-gt-imm": "sem-gt",
    }

    def _split_drain_and_barrier(self, tick_clock, wait_clock):
        nc = self.nc
        drain_inst = nc.sync.drain()
        wait_clock.add_sem_waits(
            drain_inst.ins, ScopedClock({None: tick_clock.global_clock})
        )
        si = drain_inst.ins.sync_info
        waits = list(si.on_wait) if si is not None else []
        if len(waits) > 1:
            si.on_wait = [waits[0]]
            for w in waits[1:]:
                n = nc.sync.nop(nofuse=True)
                op = _OP_FOR_MODE.get(w.wait_mode, "sem-ge")
                n.wait_op(
                    bass_rust.SemaphoreHandle(w.ant_name, w.id), w.wait_value, op
                )
        nc.all_engine_barrier()
        assert self.sems is not None
        popped = nc._tile_sem_poison_stack.pop()
        assert popped is self._sem_poison
        nc.clear_and_free_semaphores(list(self.sems.allocated().values()))
        nc.all_engine_barrier()

    tile.TileContext._drain_and_barrier = _split_drain_and_barrier
    _PATCHED = True


_ws_ctr = [0]


def _fix_waits(nc, max_waits=1):
    """Hoist excess sem waits onto InstNoOps inserted just before the
    offending instruction (same engine & block => identical semantics)."""
    import concourse.mybir as mybir

    for f in nc.m.functions:
        for b in f.blocks:
            insts = list(b.instructions)
            out, dirty = [], False
            for inst in insts:
                si = inst.sync_info
                waits = list(si.on_wait) if si is not None else []
                if len(waits) > max_waits:
                    extra = waits[:-max_waits]
                    si.on_wait = waits[-max_waits:]
                    for i in range(0, len(extra), max_waits):
                        _ws_ctr[0] += 1
                        n = mybir.InstNoOp(
                            name=f"wsplit-{_ws_ctr[0]}", engine=inst.engine
                        )
                        n.sync_info = mybir.SyncInfo(
                            on_wait=list(extra[i:i + max_waits]), on_update=[]
                        )
                        out.append(n)
                    dirty = True
                out.append(inst)
            if dirty:
                b.instructions = out


def _install_prof_shim():
    """Register the NTFF profile hook (the image's antenv lacks
    axon_hooks) so trace=True works; stub the artifact upload."""
    import sys
    import types

    if "antenv.axon_hooks" not in sys.modules:
        mod = types.ModuleType("antenv.axon_hooks")
        _hook = [None]
        mod.set_axon_ntff_profile_hook = lambda h: _hook.__setitem__(0, h)
        mod.get_axon_ntff_profile_hook = lambda: _hook[0]
        sys.modules["antenv.axon_hooks"] = mod
        import antenv

        antenv.axon_hooks = mod
    from antenv.axon_hooks import (
        get_axon_ntff_profile_hook,
        set_axon_ntff_profile_hook,
    )

    if get_axon_ntff_profile_hook() is None:
        try:
            from trn_agent_boot.trn_boot import _ntff_profile_via_ctypes

            set_axon_ntff_profile_hook(
                _ntff_profile_via_ctypes("/opt/axon/libaxon_pjrt.so")
            )
        except Exception:
            pass
    from concourse import bass_utils

    bass_utils.upload_artifacts = lambda tmpdir: f"file://{tmpdir}"


# ----------------------------------------------------------------------
# Device kernel builder (fixed structure; all data dependence is in the
# host-built tables)
# ----------------------------------------------------------------------
def _build():
    import concourse.bacc as bacc
    import concourse.mybir as mybir
    import concourse.tile as tile
    from concourse.bass import IndirectOffsetOnAxis
    from concourse.masks import make_identity

    f32 = mybir.dt.float32
    i32 = mybir.dt.int32

    nc = bacc.Bacc(None, target_bir_lowering=False)
    x = nc.declare_dram_parameter("x", [E, C, I], f32, isOutput=False)
    wT = nc.declare_dram_parameter("wT", [E, I, O], f32, isOutput=False)
    biasr = nc.declare_dram_parameter("biasr", [128, O], f32, isOutput=False)
    ptab = nc.declare_dram_parameter("ptab", [128, NT], i32, isOutput=False)
    gtab = nc.declare_dram_parameter("gtab", [128, NT], f32, isOutput=False)
    tokm = nc.declare_dram_parameter("tokm", [128, NPT], f32, isOutput=False)
    iotw = nc.declare_dram_parameter("iotw", [128, 128], f32, isOutput=False)
    outp = nc.declare_dram_parameter("out", [T, O], f32, isOutput=True)
    # bin-padded sorted contribution rows; ExternalOutput => PJRT hands the
    # NEFF a freshly zeroed donated buffer, so pad slots read back 0.
    ysrt = nc.declare_dram_parameter(
        "ysrt", [NBIN * KPAD, O], f32, isOutput=True
    )

    add = mybir.AluOpType.add
    iseq = mybir.AluOpType.is_equal

    with tile.TileContext(nc) as tc:
        with tc.tile_pool(name="const", bufs=1) as constp:
            ident = constp.tile([128, 128], f32)
            make_identity(nc, ident[:])
            wT_sb = constp.tile([128, E, O], f32)
            nc.sync.dma_start(out=wT_sb[:], in_=wT[:].rearrange("e p o -> p e o"))
            bias_sb = constp.tile([128, O], f32)
            nc.sync.dma_start(out=bias_sb[:], in_=biasr[:])
            ptab_sb = constp.tile([128, NT], i32)
            nc.sync.dma_start(out=ptab_sb[:], in_=ptab[:])
            gtab_sb = constp.tile([128, NT], f32)
            nc.sync.dma_start(out=gtab_sb[:], in_=gtab[:])
            tokm_sb = constp.tile([128, NPT], f32)
            nc.sync.dma_start(out=tokm_sb[:], in_=tokm[:])
            iotw_sb = constp.tile([128, 128], f32)
            nc.sync.dma_start(out=iotw_sb[:], in_=iotw[:])

            # ---- P1: gate*x, GEMM, +gate*bias, scatter to padded pos ----
            with tc.tile_pool(name="xw", bufs=2) as xwp, \
                 tc.tile_pool(name="xg", bufs=3) as xgp, \
                 tc.tile_pool(name="xt", bufs=3) as xtp, \
                 tc.tile_pool(name="ysb", bufs=4) as yp, \
                 tc.tile_pool(name="gb", bufs=3) as gbp, \
                 tc.tile_pool(name="pst", bufs=2, space="PSUM") as pst, \
                 tc.tile_pool(name="psy", bufs=2, space="PSUM") as psy:
                for e in range(E):
                    xw = xwp.tile([128, C // 128, I], f32)
                    nc.sync.dma_start(
                        out=xw[:], in_=x[e].rearrange("(a p) i -> p a i", p=128)
                    )
                    for ct in range(C // 128):
                        g = e * (C // 128) + ct
                        xg = xgp.tile([128, I], f32)
                        nc.vector.tensor_scalar_mul(
                            xg[:], xw[:, ct, :], gtab_sb[:, g:g + 1]
                        )
                        tp = pst.tile([128, 128], f32)
                        nc.tensor.transpose(
                            out=tp[:], in_=xg[:], identity=ident[:]
                        )
                        xT = xtp.tile([128, 128], f32)
                        nc.vector.tensor_copy(out=xT[:], in_=tp[:])
                        ypsum = psy.tile([128, O], f32)
                        nc.tensor.matmul(
                            out=ypsum[:], lhsT=xT[:], rhs=wT_sb[:, e, :],
                            start=True, stop=True,
                        )
                        gb = gbp.tile([128, O], f32)
                        nc.vector.tensor_scalar_mul(
                            gb[:], bias_sb[:], gtab_sb[:, g:g + 1]
                        )
                        ysb = yp.tile([128, O], f32)
                        nc.vector.tensor_tensor(
                            out=ysb[:], in0=ypsum[:], in1=gb[:], op=add
                        )
                        nc.gpsimd.indirect_dma_start(
                            out=ysrt[:],
                            out_offset=IndirectOffsetOnAxis(
                                ap=ptab_sb[:, g:g + 1], axis=0
                            ),
                            in_=ysb[:],
                            in_offset=None,
                        )

            # ---- P2: per-bin segment-sum via onehot matmuls (2/bin) ----
            YCH = 4  # padded k-tiles per load
            with tc.tile_pool(name="yst", bufs=3) as ystp, \
                 tc.tile_pool(name="cmp", bufs=4) as cmpp, \
                 tc.tile_pool(name="osb", bufs=3) as osbp, \
                 tc.tile_pool(name="pso", bufs=3, space="PSUM") as psop:
                psums = {}
                for gq in range(NPT // YCH):
                    yst = ystp.tile([128, YCH, O], f32)
                    nc.sync.dma_start(
                        out=yst[:],
                        in_=ysrt[gq * YCH * 128:(gq + 1) * YCH * 128, :]
                        .rearrange("(a p) o -> p a o", p=128),
                    )
                    for i in range(YCH):
                        g = gq * YCH + i
                        j = g // (KPAD // 128)
                        h = g % (KPAD // 128)
                        first = h == 0
                        last = h == KPAD // 128 - 1
                        cmp = cmpp.tile([128, 128], f32)
                        nc.vector.tensor_tensor(
                            out=cmp[:],
                            in0=tokm_sb[:, g:g + 1].to_broadcast([128, 128]),
                            in1=iotw_sb[:],
                            op=iseq,
                        )
                        if first:
                            psums[j] = psop.tile(
                                [128, O], f32, name="psum_bin", tag="psum_bin"
                            )
                        nc.tensor.matmul(
                            out=psums[j][:],
                            lhsT=cmp[:],
                            rhs=yst[:, i, :],
                            start=first, stop=last,
                        )
                        if last:
                            osb = osbp.tile([128, O], f32)
                            nc.vector.tensor_copy(out=osb[:], in_=psums[j][:])
                            nc.sync.dma_start(
                                out=outp[j * 128:(j + 1) * 128, :], in_=osb[:]
                            )
                            del psums[j]

    nc.compile()
    _fix_waits(nc)
    return nc


# ----------------------------------------------------------------------
# Host-side entry point
# ----------------------------------------------------------------------
def kernel(x_expert, expert_indices, expert_gate, weight, bias, num_tokens,
           _trace=False):
    global LAST_EXEC_NS, LAST_RESULT
    _install_patches()
    _install_prof_shim()
    from concourse.bass_utils import run_bass_kernel_spmd

    x_expert = np.ascontiguousarray(np.asarray(x_expert, dtype=np.float32))
    idx = np.asarray(expert_indices).astype(np.int64)
    gate = np.ascontiguousarray(np.asarray(expert_gate, dtype=np.float32))
    weight = np.asarray(weight, dtype=np.float32)
    bias = np.asarray(bias, dtype=np.float32)
    T_ = int(num_tokens)
    assert T_ == T and x_expert.shape == (B, E, C, I)

    wT = np.ascontiguousarray(weight.transpose(0, 2, 1))        # (E, I, O)
    biasr = np.ascontiguousarray(np.broadcast_to(bias, (128, O)))
    iotw = np.ascontiguousarray(
        np.broadcast_to(np.arange(128, dtype=np.float32), (128, 128))
    )

    per_core = []
    for b in range(B):
        fidx = idx[b].reshape(K)
        fgate = gate[b].reshape(K)
        perm = np.argsort(fidx, kind="stable")
        tok_sorted = fidx[perm]
        bin_of = tok_sorted // 128
        counts = np.bincount(bin_of, minlength=NBIN)
        if counts.max() > KPAD:
            raise RuntimeError(f"bin count {counts.max()} exceeds KPAD={KPAD}")
        # padded position of sorted row r: bin*KPAD + rank_within_bin
        starts = np.concatenate(([0], np.cumsum(counts)))[:-1]
        rank = np.arange(K) - starts[bin_of]
        padpos = (bin_of * KPAD + rank).astype(np.int64)
        sortpos = np.empty(K, dtype=np.int32)
        sortpos[perm] = padpos.astype(np.int32)
        ptab = sortpos.reshape(NT, 128).T.astype(np.int32).copy()
        gtab = fgate.reshape(NT, 128).T.astype(np.float32).copy()
        # token-minus-bin-base in padded order; pad slots get -1
        tokm_flat = np.full(NBIN * KPAD, -1.0, dtype=np.float32)
        tokm_flat[padpos] = (tok_sorted - 128 * bin_of).astype(np.float32)
        tokm = tokm_flat.reshape(NPT, 128).T.copy()
        per_core.append((ptab, gtab, tokm))

    nc = _build()
    in_maps = []
    for b in range(B):
        ptab, gtab, tokm = per_core[b]
        in_maps.append({
            "x": x_expert[b], "wT": wT, "biasr": biasr,
            "ptab": ptab, "gtab": gtab, "tokm": tokm, "iotw": iotw,
        })

    kwargs = {}
    if _trace:
        import tempfile
        kwargs = dict(trace=True, tmpdir=tempfile.mkdtemp(prefix="moe_prof_"))
    try:
        res = run_bass_kernel_spmd(
            nc, in_maps, core_ids=list(range(N_CORES)), **kwargs
        )
    except Exception:
        if not _trace:
            raise
        res = run_bass_kernel_spmd(nc, in_maps, core_ids=list(range(N_CORES)))
    LAST_EXEC_NS = res.exec_time_ns
    LAST_RESULT = res

    out = np.stack([res.results[b]["out"] for b in range(B)], axis=0)
    return out.astype(np.float32)
